# revision 22
# baseline (speedup 1.0000x reference)
"""Multi-head attention (B=2, S=2048, D=1024, H=16) on 8 NeuronCores.

Sharding: Megatron tensor parallelism. Core r owns heads 2r, 2r+1
(a 128-wide slice of D). Wq/Wk/Wv column-parallel, Wo row-parallel,
chunked ReduceScatter(add) over tokens, host reassembles and adds bo.

All matmul operands are bf16 (fp32 PSUM accumulate). Host pre-casts
activations/weights to bf16 and pre-transposes x to feature-major.

Per-core layouts:
  xqT/xkT/xvT : [1024, 4096] bf16  feature-major activations
  x tiles     : [128, 2048] per (tensor, k-tile, batch) - 24 DMAs/batch
  qT/kT       : [128, 2048] per batch; rows 0:64 = head0 dk, 64:128 = head1
  v           : [128, 130] x16 per batch; cols = [v_h0 | 1 | v_h1 | 1]
                (ones columns make the PV matmul emit softmax sums)
  scores      : psum [128 sk, 1024] = [h0 block | h1 block]; the two score
                matmuls run CONCURRENTLY via PE row tiling (K=64: h0 in
                array rows 0-63, h1 in rows 64-127)
  exp         : one ACT instr per [128, 1024] psum tile -> pt bf16 sbuf
  PV          : psum [65, 512] per head accumulated over 16 sk tiles;
                row 64 = softmax sums
  normalize   : sums -> PE broadcast (ones x sums) -> reciprocal_approx_fast
                on [128,512] (all lanes) -> one tensor_mul; no single-lane
                reciprocals on the critical path
  attnT       : [128, 2048] per batch, normalized, dk-major
  out-proj    : partial [tok, 1024] bf16 -> DRAM, ReduceScatter per token
                chunk (1024/1024/1024/512/512), overlapped with compute

The emission is software-pipelined: each (sq, sk) attention iteration
also pops one deferred thunk (previous block's normalization/out-proj,
remaining projection blocks of this batch, or the next batch's
projections) so the PE and ACT engines never drain.
"""

import sys

sys.path.insert(0, "/opt/trn_rl_repo")

import numpy as np

B, S, D, H, DK = 2, 2048, 1024, 16, 64
NCORES = 8
TOK = B * S            # 4096
DKC = D // NCORES      # 128 = 2 heads per core
KT = D // 128          # 8 contraction tiles
SKT = S // 128         # 16 key tiles per batch
SQB = S // 512         # 4 query blocks per batch

# ReduceScatter chunks: (batch, start token within batch, n tokens)
CHUNKS = [
    (0, 0, 512),
    (0, 512, 512),
    (0, 1024, 1024),
    (1, 0, 1024),
    (1, 1024, 512),
    (1, 1536, 512),
]
CHUNK_OFF = [0, 64, 128, 256, 384, 448]  # row offset per chunk in out_ext

_cache = {}


def _build(collective=True):
    from contextlib import ExitStack

    from concourse import bacc
    import concourse.mybir as mybir
    import concourse.tile as tile

    f32 = mybir.dt.float32
    bf16 = mybir.dt.bfloat16
    Act = mybir.ActivationFunctionType

    nc = bacc.Bacc(
        "TRN2", target_bir_lowering=False, debug=False,
        enable_asserts=False, num_devices=NCORES,
    )

    # x tensors host-arranged as [8, 128, 4096]: row block b*4+q holds
    # tokens [b*2048+q*512 : +512), cols = k-tile * 512 + token offset
    xqT = nc.dram_tensor("xqT", [1024, 4096], bf16, kind="ExternalInput").ap()
    xkT = nc.dram_tensor("xkT", [1024, 4096], bf16, kind="ExternalInput").ap()
    xvT = nc.dram_tensor("xvT", [1024, 4096], bf16, kind="ExternalInput").ap()
    # weights host-arranged as [128, 8*128]: k-tiles side by side
    wq = nc.dram_tensor("wq", [128, D], bf16, kind="ExternalInput").ap()
    wk = nc.dram_tensor("wk", [128, D], bf16, kind="ExternalInput").ap()
    wv = nc.dram_tensor("wv", [128, D], bf16, kind="ExternalInput").ap()
    wo = nc.dram_tensor("wo", [DKC, D], bf16, kind="ExternalInput").ap()
    bq = nc.dram_tensor("bq", [DKC, 1], f32, kind="ExternalInput").ap()
    bk = nc.dram_tensor("bk", [DKC, 1], f32, kind="ExternalInput").ap()
    bv = nc.dram_tensor("bv", [1, DKC], bf16, kind="ExternalInput").ap()
    out_ext = nc.dram_tensor("out", [512, D], bf16, kind="ExternalOutput").ap()

    with tile.TileContext(nc) as tc, ExitStack() as ctx, \
            nc.allow_low_precision("bf16 matmul operands, fp32 psum accumulate"):
        wpool = ctx.enter_context(tc.tile_pool(name="w", bufs=1))
        xpool = ctx.enter_context(tc.tile_pool(name="x", bufs=5))
        qkpool = ctx.enter_context(tc.tile_pool(name="qk", bufs=2))
        vpool = ctx.enter_context(tc.tile_pool(name="v", bufs=36))
        ptpool = ctx.enter_context(tc.tile_pool(name="pt", bufs=5))
        atpool = ctx.enter_context(tc.tile_pool(name="at", bufs=2))
        smpool = ctx.enter_context(tc.tile_pool(name="sm", bufs=2))
        opool = ctx.enter_context(tc.tile_pool(name="o", bufs=3))
        # PSUM budget (8 banks): sc 2x[128,1024]=4, acc0+acc1=2, gen 2x[128,512]=2
        ps_sc = ctx.enter_context(tc.tile_pool(name="pssc", bufs=2, space="PSUM"))
        ps_acc = ctx.enter_context(tc.tile_pool(name="psacc", bufs=1, space="PSUM"))
        ps_gen = ctx.enter_context(tc.tile_pool(name="psgen", bufs=2, space="PSUM"))
        dram = ctx.enter_context(tc.tile_pool(name="dram", bufs=1, space="DRAM"))

        # ---- constants / weights into SBUF (one DMA per weight) ----
        wq_a = wpool.tile([128, D], bf16, tag="wqa")
        nc.sync.dma_start(wq_a[:], wq[:])
        wk_a = wpool.tile([128, D], bf16, tag="wka")
        nc.sync.dma_start(wk_a[:], wk[:])
        wv_a = wpool.tile([128, D], bf16, tag="wva")
        nc.sync.dma_start(wv_a[:], wv[:])
        wq_t = [wq_a[:, k * 128:(k + 1) * 128] for k in range(KT)]
        wk_t = [wk_a[:, k * 128:(k + 1) * 128] for k in range(KT)]
        wv_t = [wv_a[:, k * 128:(k + 1) * 128] for k in range(KT)]
        wo_t = wpool.tile([DKC, D], bf16, tag="wo")
        nc.sync.dma_start(wo_t[:], wo[:])
        bq_t = wpool.tile([DKC, 1], f32, tag="bq")
        nc.sync.dma_start(bq_t[:], bq[:])
        bk_t = wpool.tile([DKC, 1], f32, tag="bk")
        nc.sync.dma_start(bk_t[:], bk[:])
        bv_t = wpool.tile([1, DKC], bf16, tag="bv")
        nc.sync.dma_start(bv_t[:], bv[:])
        ones_t = wpool.tile([1, 128], bf16, tag="ones")
        nc.vector.memset(ones_t[:], 1.0)

        partials = [dram.tile([n, D], bf16, tag=f"partial{c}",
                              name=f"partial{c}")
                    for c, (_, _, n) in enumerate(CHUNKS)]
        rs_outs = [dram.tile([n // NCORES, D], bf16, tag=f"rsout{c}",
                             name=f"rsout{c}")
                   for c, (_, _, n) in enumerate(CHUNKS)]

        # ---------- emission helpers ----------

        def load_x_q(xT, b, q, tag):
            """One DMA for a [128, 4096] block = one 512-token sq block."""
            xt = xpool.tile([128, 4096], bf16, tag=tag, name=f"{tag}{b}_{q}")
            r0 = (b * 4 + q) * 128
            nc.sync.dma_start(xt[:], xT[r0:r0 + 128, :])
            return xt

        def qk_block(xts, w_list, bias_t, dst, blk):
            """One 512-col block of a q/k projection -> dst[:, blk]."""
            xt = xts[blk]
            ps = ps_gen.tile([128, 512], f32, tag="gen", name="psqk")
            for k in range(KT):
                nc.tensor.matmul(
                    ps[:], lhsT=w_list[k], rhs=xt[:, k * 512:(k + 1) * 512],
                    start=(k == 0), stop=(k == KT - 1),
                )
            nc.vector.tensor_scalar_add(
                dst[:, blk * 512:(blk + 1) * 512], ps[:], bias_t[:, 0:1])

        def v_block(xts, blk, v_tiles, mis=(0, 1, 2, 3)):
            """Token-tiles [128, 130] of the v projection (subset mis)."""
            xt = xts[blk]
            for mi in mis:
                ps = ps_gen.tile([128, 512], f32, tag="gen", name="psv")
                for k in range(KT):
                    nc.tensor.matmul(
                        ps[:, 0:128],
                        lhsT=xt[:, k * 512 + mi * 128:k * 512 + mi * 128 + 128],
                        rhs=wv_t[k], start=(k == 0), stop=False,
                    )
                nc.tensor.matmul(
                    ps[:, 0:128], lhsT=ones_t[0:1, :], rhs=bv_t[:],
                    start=False, stop=True,
                )
                vt = vpool.tile([128, 130], bf16, tag="v")
                nc.vector.tensor_copy(vt[:, 0:64], ps[:, 0:64])
                nc.vector.tensor_copy(vt[:, 65:129], ps[:, 64:128])
                nc.vector.memset(vt[:, 64:65], 1.0)
                nc.vector.memset(vt[:, 129:130], 1.0)
                v_tiles.append(vt)

        def emit_scores_exp(qT_b, kT_b, sq, sk):
            """Packed scores pair + one exp -> pt tile."""
            qs = slice(sq * 512, (sq + 1) * 512)
            ks = slice(sk * 128, (sk + 1) * 128)
            sps = ps_sc.tile([128, 1024], f32, tag="sc")
            # two heads run concurrently in PE rows 0-63 / 64-127
            nc.tensor.matmul(
                sps[:, 0:512], lhsT=kT_b[0:64, ks], rhs=qT_b[0:64, qs],
                start=True, stop=True,
            )
            nc.tensor.matmul(
                sps[:, 512:1024], lhsT=kT_b[64:128, ks], rhs=qT_b[64:128, qs],
                start=True, stop=True,
            )
            pt = ptpool.tile([128, 1024], bf16, tag="pt")
            nc.scalar.activation(pt[:], sps[:], Act.Exp, scale=0.125)
            return pt

        def emit_pv(v_tiles, accs, pt, sk):
            nc.tensor.matmul(
                accs[0][:], lhsT=v_tiles[sk][:, 0:65], rhs=pt[:, 0:512],
                start=(sk == 0), stop=(sk == SKT - 1),
            )
            nc.tensor.matmul(
                accs[1][:], lhsT=v_tiles[sk][:, 65:130], rhs=pt[:, 512:1024],
                start=(sk == 0), stop=(sk == SKT - 1),
            )

        def drain_accs(accs):
            """Inline epilogue part 1 (cheap DVE copies only): pull sums and
            raw attention out of PSUM so the acc banks free up fast."""
            sums = smpool.tile([1, 1024], bf16, tag="sums")
            araw = smpool.tile([128, 512], bf16, tag="araw")
            nc.vector.tensor_copy(sums[0:1, 0:512], accs[0][64:65, :])
            nc.vector.tensor_copy(sums[0:1, 512:1024], accs[1][64:65, :])
            nc.vector.tensor_copy(araw[0:64, :], accs[0][0:64, :])
            nc.vector.tensor_copy(araw[64:128, :], accs[1][0:64, :])
            return sums, araw

        def make_epilogue(attnT_b, sums, araw, b, sq):
            """Deferred epilogue thunks for (b, sq): broadcast-normalize,
            out-projection, and (if chunk-final) the chunk's ReduceScatter."""
            qs = slice(sq * 512, (sq + 1) * 512)

            def s_norm():
                rbs_ps = ps_gen.tile([128, 512], f32, tag="gen", name="rbs_ps")
                # the two broadcast matmuls run concurrently (col tiling)
                nc.tensor.matmul(
                    rbs_ps[0:64, :], lhsT=ones_t[0:1, 0:64],
                    rhs=sums[0:1, 0:512], start=True, stop=True,
                )
                nc.tensor.matmul(
                    rbs_ps[64:128, :], lhsT=ones_t[0:1, 0:64],
                    rhs=sums[0:1, 512:1024], start=True, stop=True,
                )
                rb = smpool.tile([128, 512], f32, tag="rb")
                nc.vector.reciprocal_approx_fast(rb[:], rbs_ps[:])
                nc.vector.tensor_mul(attnT_b[:, qs], araw[:], rb[:])

            def find_chunk(tok):
                return next(c for c, (bb, t0, n) in enumerate(CHUNKS)
                            if bb == b and t0 <= tok < t0 + n)

            def make_rs(chunk):
                rows = CHUNKS[chunk][2] // NCORES
                off = CHUNK_OFF[chunk]

                def rs():
                    nc.gpsimd.collective_compute(
                        "ReduceScatter",
                        mybir.AluOpType.add,
                        replica_groups=[list(range(NCORES))],
                        ins=[partials[chunk].opt()],
                        outs=[rs_outs[chunk].opt()],
                    )
                    nc.gpsimd.dma_start(
                        out_ext[off:off + rows, :], rs_outs[chunk][:])
                return rs

            def out_m(m):
                def f():
                    col = sq * 512 + m * 128
                    tok = sq * 512 + m * 128
                    chunk = find_chunk(tok)
                    srow = tok - CHUNKS[chunk][1]
                    for n2 in range(2):
                        ops = ps_gen.tile([128, 512], f32, tag="gen",
                                          name="psout")
                        nc.tensor.matmul(
                            ops[:], lhsT=attnT_b[:, col:col + 128],
                            rhs=wo_t[:, n2 * 512:(n2 + 1) * 512],
                            start=True, stop=True,
                        )
                        ot = opool.tile([128, 512], bf16, tag="ot")
                        nc.vector.tensor_copy(ot[:], ops[:])
                        nc.sync.dma_start(
                            partials[chunk][srow:srow + 128,
                                            n2 * 512:(n2 + 1) * 512],
                            ot[:],
                        )
                return f

            thunks = [s_norm]
            for m in range(4):
                thunks.append(out_m(m))
                tok = sq * 512 + m * 128
                chunk = find_chunk(tok)
                if collective and tok + 128 == CHUNKS[chunk][1] + CHUNKS[chunk][2]:
                    thunks.append(make_rs(chunk))
            return thunks

        # ---------- main emission ----------
        qT, kT, attnT = {}, {}, {}
        vt_all = {0: [], 1: []}
        for b in (0, 1):
            kT[b] = qkpool.tile([128, S], bf16, tag="kT", name=f"kT{b}")
            qT[b] = qkpool.tile([128, S], bf16, tag="qT", name=f"qT{b}")
            attnT[b] = atpool.tile([128, S], bf16, tag="attnT",
                                   name=f"attnT{b}")

        # batch 0: first-half x DMAs land before anything else so the
        # first projection blocks can start ~immediately
        xb0 = {t: [None] * SQB for t in ("xk", "xq", "xv")}
        xb1 = {t: [None] * SQB for t in ("xk", "xq", "xv")}
        for q in range(SQB):
            for tag, xT in (("xk", xkT), ("xq", xqT), ("xv", xvT)):
                xb0[tag][q] = load_x_q(xT, 0, q, tag)
        qk_block(xb0["xk"], wk_t, bk_t, kT[0], 0)
        qk_block(xb0["xq"], wq_t, bq_t, qT[0], 0)
        v_block(xb0["xv"], 0, vt_all[0])

        def b1_dma(name, xT, q):
            def f():
                xb1[name][q] = load_x_q(xT, 1, q, name)
            return f

        # the rest of b0's projections + all of b1's become deferred thunks;
        # v blocks split in halves so no single thunk floods the PE
        extra = [
            lambda: qk_block(xb0["xk"], wk_t, bk_t, kT[0], 1),
            lambda: v_block(xb0["xv"], 1, vt_all[0], (0, 1)),
            lambda: v_block(xb0["xv"], 1, vt_all[0], (2, 3)),
            lambda: qk_block(xb0["xq"], wq_t, bq_t, qT[0], 1),
            lambda: qk_block(xb0["xk"], wk_t, bk_t, kT[0], 2),
            lambda: v_block(xb0["xv"], 2, vt_all[0], (0, 1)),
            lambda: v_block(xb0["xv"], 2, vt_all[0], (2, 3)),
            lambda: qk_block(xb0["xq"], wq_t, bq_t, qT[0], 2),
            lambda: qk_block(xb0["xk"], wk_t, bk_t, kT[0], 3),
            lambda: v_block(xb0["xv"], 3, vt_all[0], (0, 1)),
            lambda: v_block(xb0["xv"], 3, vt_all[0], (2, 3)),
            lambda: qk_block(xb0["xq"], wq_t, bq_t, qT[0], 3),
            b1_dma("xk", xkT, 0),
            b1_dma("xq", xqT, 0),
            b1_dma("xv", xvT, 0),
            b1_dma("xk", xkT, 1),
            b1_dma("xq", xqT, 1),
            b1_dma("xv", xvT, 1),
            b1_dma("xk", xkT, 2),
            b1_dma("xq", xqT, 2),
            b1_dma("xv", xvT, 2),
            b1_dma("xk", xkT, 3),
            b1_dma("xq", xqT, 3),
            b1_dma("xv", xvT, 3),
        ]
        for blk in range(SQB):
            extra.append(lambda blk=blk: qk_block(
                xb1["xk"], wk_t, bk_t, kT[1], blk))
        for blk in range(SQB):
            extra.append(lambda blk=blk: qk_block(
                xb1["xq"], wq_t, bq_t, qT[1], blk))
        for blk in range(SQB):
            extra.append(lambda blk=blk: v_block(
                xb1["xv"], blk, vt_all[1], (0, 1)))
            extra.append(lambda blk=blk: v_block(
                xb1["xv"], blk, vt_all[1], (2, 3)))

        pending = []
        prev = [None]  # (accs, batch, sq) awaiting drain

        def flush_prev():
            if prev[0] is None:
                return
            paccs, pb, psq = prev[0]
            prev[0] = None
            with tc.high_priority(offset=1_000_000):
                sums, araw = drain_accs(paccs)
            pending.extend(make_epilogue(attnT[pb], sums, araw, pb, psq))

        def emit_batch_attention(b):
            for sq in range(SQB):
                # iteration 0 emits scores+exp only (PV is deferred), so the
                # previous block's drain overlaps exp(sq, 0) and ACT never
                # pauses at the boundary
                pt0 = emit_scores_exp(qT[b], kT[b], sq, 0)
                flush_prev()
                accs = (
                    ps_acc.tile([65, 512], f32, tag="acc0", name="acc0"),
                    ps_acc.tile([65, 512], f32, tag="acc1", name="acc1"),
                )
                held = (pt0, 0)
                for sk in range(1, SKT):
                    pt = emit_scores_exp(qT[b], kT[b], sq, sk)
                    emit_pv(vt_all[b], accs, *held)
                    held = (pt, sk)
                    if sk >= SKT - 2:
                        continue  # keep the boundary clean for the drain
                    if pending:
                        pending.pop(0)()
                        if pending:  # epilogue thunks are light; pop two
                            pending.pop(0)()
                    elif extra:
                        extra.pop(0)()
                emit_pv(vt_all[b], accs, *held)
                prev[0] = (accs, b, sq)

        emit_batch_attention(0)
        # all of b1's projection thunks must be emitted before b1's
        # attention reads their outputs (deps are tracked in program order)
        while extra:
            extra.pop(0)()
        emit_batch_attention(1)
        flush_prev()
        for f in pending:
            f()
        pending.clear()

        if not collective:
            for c, (_, _, rows) in enumerate(CHUNKS):
                nc.sync.dma_start(
                    out_ext[CHUNK_OFF[c]:CHUNK_OFF[c] + rows, :],
                    partials[c][0:rows, :],
                )

    nc.compile()
    return nc


def _get_nc():
    if "nc" not in _cache:
        _cache["nc"] = _build()
    return _cache["nc"]


def kernel(query, key, value, Wq, bq, Wk, bk, Wv, bv, Wo, bo, trace=False):
    from concourse.bass_utils import run_bass_kernel_spmd
    import ml_dtypes

    bfloat16 = ml_dtypes.bfloat16
    nc = _get_nc()

    def arrange_x(x):
        # [TOK, D] -> [8, 128, 4096]: row block b*4+q holds tokens
        # [b*2048+q*512 : +512), cols = k-tile * 512 + token offset
        x = np.asarray(x, np.float32).reshape(2, 4, 512, 8, 128)
        return np.ascontiguousarray(
            x.transpose(0, 1, 4, 3, 2).reshape(1024, 4096).astype(bfloat16))

    def arrange_w(w):
        # [D, 128] -> [128, 8*128]: k-tiles side by side
        return np.ascontiguousarray(
            w.reshape(8, 128, 128).transpose(1, 0, 2).reshape(128, 1024)
            .astype(bfloat16))

    q = arrange_x(query)
    k = arrange_x(key)
    v = arrange_x(value)
    Wq = np.asarray(Wq, np.float32)
    Wk = np.asarray(Wk, np.float32)
    Wv = np.asarray(Wv, np.float32)
    Wo = np.asarray(Wo, np.float32)

    in_maps = []
    for r in range(NCORES):
        sl = slice(r * DKC, (r + 1) * DKC)
        in_maps.append({
            "xqT": q, "xkT": k, "xvT": v,
            "wq": arrange_w(Wq[:, sl]),
            "wk": arrange_w(Wk[:, sl]),
            "wv": arrange_w(Wv[:, sl]),
            "wo": np.ascontiguousarray(Wo[sl, :].astype(bfloat16)),
            "bq": np.ascontiguousarray(np.asarray(bq, np.float32)[sl, None]),
            "bk": np.ascontiguousarray(np.asarray(bk, np.float32)[sl, None]),
            "bv": np.ascontiguousarray(
                np.asarray(bv, np.float32)[None, sl].astype(bfloat16)),
        })

    res = run_bass_kernel_spmd(nc, in_maps, list(range(NCORES)), trace=trace)
    _cache["last_results"] = res

    # Reassemble: chunk c scatters its rows over cores; core r's piece of
    # chunk c sits at out_ext[CHUNK_OFF[c] : +rows].
    out = np.zeros((TOK, D), np.float32)
    for r in range(NCORES):
        o = np.asarray(res.results[r]["out"]).astype(np.float32)
        for c, (b, tok0, n) in enumerate(CHUNKS):
            rows = n // NCORES
            t0 = b * S + tok0 + r * rows
            off = CHUNK_OFF[c]
            out[t0:t0 + rows] = o[off:off + rows]
    out = out + np.asarray(bo, np.float32)[None, :]
    return out.reshape(B, S, D)
